# revision 26
# baseline (speedup 1.0000x reference)
"""v3 feature kernel: dual-player packed planes (my: bits 0-7, op: bits 16-23),
2-stage board pipeline so output DMA streams during logic of the next group.

Key changes vs v2:
- Expansion: per-j fused tensor_scalar ((Rg >> j|16+j) & 1 -> f32) straight
  from the packed channel planes; no mask tile, no mask-AND pass, no IS_NE
  pass. TS-class ops run at 2 partitions/cycle on DVE.
- Doubles: SWAR popcount on the packed words + log-step row adds; no f32
  tensor_reduce over expanded channels.
- Engine split: V = is_eq/pack/planes/line-row/conn dirs01/merges/SWAR/exp-my;
  GpSimd = line-col/conn dirs23/exp-op; Act = doubles broadcast.
- Output DMA per channel group (ch0-1 early, conn ch2-7, ch8-12, ch13-17)
  per pipe; 2 pipes of 16 boards pipeline logic with expansion DMA.
"""
import numpy as np

import concourse.bass as bass
import concourse.bacc as bacc
import concourse.mybir as mybir
import concourse.tile as tile

Alu = mybir.AluOpType
Act = mybir.ActivationFunctionType
DT = mybir.dt

P = 128
NB = 32
CB = 8             # boards per expansion chunk
NCORES = 8
BPC = P * NB
PAD = 18
R0 = 5
ROWS = slice(R0, R0 + 8)
SEG = 0x00FF00FF
PADT = 12
R0T = 2

DIRS = ((0, 1), (1, 0), (1, 1), (1, -1))


def _stt_raw(eng, out, in0, imm, in1, op0, op1, imm_dt=DT.uint32):
    outs = [eng.lower_ap(out)]
    return eng.add_instruction(
        mybir.InstTensorScalarPtr(
            name=eng.bass.get_next_instruction_name(),
            is_scalar_tensor_tensor=True,
            op0=op0, op1=op1,
            ins=[eng.lower_ap(in0),
                 mybir.ImmediateValue(dtype=imm_dt, value=imm),
                 eng.lower_ap(in1)],
            outs=outs,
        )
    )


def _stt(eng, out, in0, sh, op1, in1):
    if sh > 0:
        _stt_raw(eng, out, in0, sh, in1, Alu.logical_shift_left, op1)
    elif sh < 0:
        _stt_raw(eng, out, in0, -sh, in1, Alu.logical_shift_right, op1)
    else:
        eng.tensor_tensor(out, in0, in1, op1)


def feature_kernel(tc, out_d, state_d, side_d):
    nc = tc.nc
    V, A = nc.vector, nc.scalar

    state_v = state_d.rearrange("(p n) c -> p n c", p=P)
    side_v = side_d.rearrange("(p n) -> p n", p=P)
    out_v = out_d.rearrange("(p n) c -> p n c", p=P)

    with (
        tc.tile_pool(name="main", bufs=1) as pool,
        tc.tile_pool(name="exp", bufs=2) as epool,
    ):
        # ---------- input ----------
        sideT = pool.tile([P, NB], DT.float32, name="sideT")
        nc.sync.dma_start(sideT[:], side_v)
        negside = pool.tile([P, NB], DT.float32, name="negside")
        V.tensor_scalar(negside[:], sideT[:], -1.0, None, Alu.mult)

        mo = pool.tile([P, NB, 2, 64], DT.float32, name="mo")   # ch0/1 planes
        Rg = pool.tile([P, 6, NB, 8], DT.uint32, name="Rg")     # packed channels
        dge = pool.tile([P, 4, NB], DT.float32, name="dge")     # doubles flags
        myR = pool.tile([P, NB, 8], DT.uint32, name="myR")
        opR = pool.tile([P, NB, 8], DT.uint32, name="opR")

        # tiles whose guard-zeroing can overlap the input DMA
        P4 = pool.tile([P, 4, NB, PAD], DT.uint32, name="P4")
        V.memset(P4[:, :, :, 0:R0], 0)
        V.memset(P4[:, :, :, R0 + 8:PAD], 0)
        NAMES = ("t", "u", "a", "w", "b", "y", "q", "l2", "m3", "r1", "c",
                 "i1", "l3", "lb", "d0", "d1", "d", "j1", "md", "o3", "rb",
                 "x", "e", "g1", "g2", "r3", "c_l2", "c_l3", "c_r3")
        TMP = pool.tile([P, len(NAMES), NB, PADT], DT.uint32, name="TMP")
        V.memset(TMP[:, :, :, 0:R0T], 0)
        V.memset(TMP[:, :, :, R0T + 8:PADT], 0)
        lbm = pool.tile([P, NB, 8], DT.uint32, name="lbm")
        rbm = pool.tile([P, NB, 8], DT.uint32, name="rbm")
        V.memset(lbm[:], 0)
        V.memset(lbm[:, :, 0:1], SEG)
        V.memset(rbm[:], 0)
        V.memset(rbm[:, :, 3:8], SEG)
        CV = pool.tile([P, 5, NB, PAD], DT.uint32, name="CV")
        V.memset(CV[:, :, :, R0 - 1:R0], 0)
        V.memset(CV[:, :, :, R0 + 8:R0 + 10], 0)

        with tc.tile_pool(name="pre", bufs=1) as ppre:
            s = ppre.tile([P, NB, 64], DT.float32, name="s")
            nc.sync.dma_start(s[:], state_v)
            pk1 = ppre.tile([P, NB, 8, 4], DT.float32, name="pk1")
            pk2 = ppre.tile([P, NB, 8, 2], DT.float32, name="pk2")

            V.tensor_tensor(
                mo[:, :, 0], s[:],
                sideT[:, :, None].broadcast_to((P, NB, 64)), Alu.is_equal)
            V.tensor_tensor(
                mo[:, :, 1], s[:],
                negside[:, :, None].broadcast_to((P, NB, 64)), Alu.is_equal)
            nc.sync.dma_start(
                out_v[:, :, 0:128],
                mo.rearrange("p n c x -> p n (c x)"))

            def pack(dst_ap, srcf):
                v = srcf.rearrange("p n (r j2 t) -> p n r j2 t", t=2, j2=4)
                a1, b1 = v[:, :, :, :, 1], v[:, :, :, :, 0]
                V.scalar_tensor_tensor(pk1[:], a1, 2.0, b1, op0=Alu.mult,
                                       op1=Alu.add)
                ww2 = pk1.rearrange("p n r (k t) -> p n r k t", t=2)
                a2, b2 = ww2[:, :, :, :, 1], ww2[:, :, :, :, 0]
                V.scalar_tensor_tensor(pk2[:], a2, 4.0, b2, op0=Alu.mult,
                                       op1=Alu.add)
                a3, b3 = pk2[:, :, :, 1], pk2[:, :, :, 0]
                V.scalar_tensor_tensor(dst_ap, a3, 16.0, b3, op0=Alu.mult,
                                       op1=Alu.add)

            pack(myR[:], mo[:, :, 0])
            pack(opR[:], mo[:, :, 1])

        # ---------- planes ----------
        Ad, Bd, Ed, Nd = (P4[:, i] for i in range(4))
        _stt(V, Ad[:, :, ROWS], opR[:], 16, Alu.bitwise_or, myR[:])
        _stt(V, Bd[:, :, ROWS], myR[:], 16, Alu.bitwise_or, opR[:])
        V.tensor_tensor(Ed[:, :, ROWS], Ad[:, :, ROWS], Bd[:, :, ROWS],
                        Alu.bitwise_or)
        V.tensor_scalar(Ed[:, :, ROWS], Ed[:, :, ROWS], SEG, None,
                        Alu.bitwise_xor)
        V.tensor_scalar(Nd[:, :, ROWS], Ad[:, :, ROWS], SEG, None,
                        Alu.bitwise_xor)

        # line-feature tmps, shared between row and col mode (V-serial)
        nix = {n: i for i, n in enumerate(NAMES)}

        def TT(n, k=0):
            return TMP[:, nix[n], :, R0T + k:R0T + 8 + k]

        # ---------- line features (rows then cols, shared tmps) ----------
        def line_feats_row():
            me, op, em, nm = (x[:, :, ROWS] for x in (Ad, Bd, Ed, Nd))
            T = TT
            _stt(V, T("t"), me, -1, Alu.bitwise_and, me)
            _stt(V, T("u"), em, -1, Alu.bitwise_and, em)
            _stt(V, T("a"), T("u"), -2, Alu.bitwise_and, T("t"))
            _stt(V, T("w"), em, -3, Alu.bitwise_and, em)
            _stt(V, T("b"), T("t"), -1, Alu.bitwise_and, T("w"))
            _stt(V, T("y"), T("b"), 1, Alu.bitwise_or, T("b"))
            V.tensor_tensor(T("q"), T("a"), T("y"), Alu.bitwise_or)
            _stt(V, T("l2"), T("q"), 1, Alu.bitwise_or, T("a"))

            _stt(V, T("m3"), me, -2, Alu.bitwise_and, T("t"))
            _stt(V, T("r1"), em, -4, Alu.bitwise_and, em)
            _stt(V, T("c"), T("m3"), -1, Alu.bitwise_and, T("r1"))
            _stt(V, T("i1"), T("c"), 1, Alu.bitwise_or, T("c"))
            _stt(V, T("l3"), T("i1"), 1, Alu.bitwise_or, T("c"))

            V.tensor_scalar(T("lb"), op, 1, 0x00010001,
                            op0=Alu.logical_shift_left, op1=Alu.bitwise_or)
            _stt(V, T("d0"), em, -3, Alu.bitwise_and, T("m3"))
            _stt(V, T("d1"), nm, -4, Alu.bitwise_and, T("d0"))
            V.tensor_tensor(T("d"), T("d1"), T("lb"), Alu.bitwise_and)
            _stt(V, T("j1"), T("d"), 1, Alu.bitwise_or, T("d"))
            _stt(V, T("md"), T("d"), 2, Alu.bitwise_or, T("j1"))
            _stt(V, T("o3"), T("m3"), -1, Alu.bitwise_and, nm)
            _stt(V, T("o3"), nm, -4, Alu.bitwise_and, T("o3"))
            V.tensor_scalar(T("rb"), op, 5, 0x00F800F8,
                            op0=Alu.logical_shift_right, op1=Alu.bitwise_or)
            V.tensor_tensor(T("x"), T("lb"), T("rb"), Alu.bitwise_xor)
            V.tensor_tensor(T("e"), T("o3"), T("x"), Alu.bitwise_and)
            _stt(V, T("g1"), T("e"), 1, Alu.bitwise_or, T("e"))
            _stt(V, T("g2"), T("g1"), 1, Alu.bitwise_or, T("e"))
            _stt(V, T("r3"), T("g2"), 1, Alu.bitwise_or, T("md"))

        def line_feats_col():
            def dn(x, k):
                return x[:, :, R0 + k:R0 + 8 + k]

            def T(n, k=0):
                nm = "c_" + n if n in ("l2", "l3", "r3") else n
                return TT(nm, k)

            me, op, em, nm = Ad, Bd, Ed, Nd

            def MV(x, k=0):
                return x[:, :, R0 + k:R0 + 8 + k]

            V.tensor_tensor(T("t"), MV(me), dn(me, 1), Alu.bitwise_and)
            V.tensor_tensor(T("u"), MV(em), dn(em, 1), Alu.bitwise_and)
            V.tensor_tensor(T("a"), T("t"), T("u", 2), Alu.bitwise_and)
            V.tensor_tensor(T("w"), MV(em), dn(em, 3), Alu.bitwise_and)
            V.tensor_tensor(T("b"), T("w"), T("t", 1), Alu.bitwise_and)
            V.tensor_tensor(T("y"), T("b"), T("b", -1), Alu.bitwise_or)
            V.tensor_tensor(T("q"), T("a"), T("y"), Alu.bitwise_or)
            V.tensor_tensor(T("l2"), T("a"), T("q", -1), Alu.bitwise_or)

            V.tensor_tensor(T("m3"), T("t"), dn(me, 2), Alu.bitwise_and)
            V.tensor_tensor(T("r1"), MV(em), dn(em, 4), Alu.bitwise_and)
            V.tensor_tensor(T("c"), T("r1"), T("m3", 1), Alu.bitwise_and)
            V.tensor_tensor(T("i1"), T("c"), T("c", -1), Alu.bitwise_or)
            V.tensor_tensor(T("l3"), T("c"), T("i1", -1), Alu.bitwise_or)

            V.tensor_tensor(T("lb"), MV(op, -1), lbm[:], Alu.bitwise_or)
            V.tensor_tensor(T("d0"), T("m3"), dn(em, 3), Alu.bitwise_and)
            V.tensor_tensor(T("d1"), T("d0"), dn(nm, 4), Alu.bitwise_and)
            V.tensor_tensor(T("d"), T("d1"), T("lb"), Alu.bitwise_and)
            V.tensor_tensor(T("j1"), T("d"), T("d", -1), Alu.bitwise_or)
            V.tensor_tensor(T("md"), T("j1"), T("d", -2), Alu.bitwise_or)
            V.tensor_tensor(T("o3"), T("m3", 1), MV(nm), Alu.bitwise_and)
            V.tensor_tensor(T("o3"), T("o3"), dn(nm, 4), Alu.bitwise_and)
            V.tensor_tensor(T("rb"), MV(op, 5), rbm[:], Alu.bitwise_or)
            V.tensor_tensor(T("x"), T("lb"), T("rb"), Alu.bitwise_xor)
            V.tensor_tensor(T("e"), T("o3"), T("x"), Alu.bitwise_and)
            V.tensor_tensor(T("g1"), T("e"), T("e", -1), Alu.bitwise_or)
            V.tensor_tensor(T("g2"), T("e"), T("g1", -1), Alu.bitwise_or)
            V.tensor_tensor(T("r3"), T("md"), T("g2", -1), Alu.bitwise_or)

        line_feats_row()
        line_feats_col()

        V.tensor_tensor(Rg[:, 3], TT("l2"), TT("c_l2"), Alu.bitwise_or)
        _stt(V, Rg[:, 4], TT("l3"), 1, Alu.bitwise_or, TT("c_l3", -1))
        V.tensor_tensor(Rg[:, 5], TT("r3"), TT("c_r3"), Alu.bitwise_or)

        # ---------- doubles via SWAR popcount on packed planes ----------
        w1 = pool.tile([P, 3, NB, 8], DT.uint32, name="w1")
        w2 = pool.tile([P, 3, NB, 8], DT.uint32, name="w2")
        cs23 = pool.tile([P, 4, NB], DT.uint32, name="cs23")
        V.tensor_scalar(w1[:], Rg[:, 3:6], 1, 0x00550055,
                        op0=Alu.logical_shift_right, op1=Alu.bitwise_and)
        V.tensor_tensor(w1[:], Rg[:, 3:6], w1[:], Alu.subtract)
        V.tensor_scalar(w2[:], w1[:], 2, 0x00330033,
                        op0=Alu.logical_shift_right, op1=Alu.bitwise_and)
        V.tensor_scalar(w1[:], w1[:], 0x00330033, None, Alu.bitwise_and)
        V.tensor_tensor(w1[:], w1[:], w2[:], Alu.add)
        V.tensor_scalar(w2[:], w1[:], 4, None, Alu.logical_shift_right)
        V.tensor_tensor(w1[:], w1[:], w2[:], Alu.add)
        V.tensor_scalar(w1[:], w1[:], 0x000F000F, None, Alu.bitwise_and)
        V.tensor_tensor(w1[:, :, :, 0:4], w1[:, :, :, 0:4],
                        w1[:, :, :, 4:8], Alu.add)
        V.tensor_tensor(w1[:, :, :, 0:2], w1[:, :, :, 0:2],
                        w1[:, :, :, 2:4], Alu.add)
        V.tensor_tensor(cs23[:, 0:3], w1[:, :, :, 0], w1[:, :, :, 1],
                        Alu.add)
        V.tensor_tensor(cs23[:, 3], cs23[:, 1], cs23[:, 2], Alu.add)
        V.tensor_scalar(cs23[:, 1], cs23[:, 0], 0xFF, None, Alu.bitwise_and)
        V.tensor_scalar(cs23[:, 2], cs23[:, 3], 0xFF, None, Alu.bitwise_and)
        V.tensor_scalar(cs23[:, 0], cs23[:, 0], 16, None,
                        Alu.logical_shift_right)
        V.tensor_scalar(cs23[:, 3], cs23[:, 3], 16, None,
                        Alu.logical_shift_right)
        V.tensor_scalar(dge[:, 0], cs23[:, 1], 2, None, Alu.is_ge)
        V.tensor_scalar(dge[:, 1], cs23[:, 2], 2, None, Alu.is_ge)
        V.tensor_scalar(dge[:, 2], cs23[:, 0], 2, None, Alu.is_ge)
        V.tensor_scalar(dge[:, 3], cs23[:, 3], 2, None, Alu.is_ge)

        # ---------- expansion helpers ----------
        mk = pool.tile([P, CB, 3, 64], DT.uint32, name="mk")
        mk16 = mk[:].bitcast(DT.uint16)

        def expand(lanes, n0, cb, out_my, out_op):
            Rv = Rg[:, lanes, n0:n0 + cb].rearrange("p c n r -> p n c r")
            for j in range(8):
                V.tensor_scalar(mk[:, 0:cb, :, j::8], Rv, j, 0x00010001,
                                op0=Alu.logical_shift_right,
                                op1=Alu.bitwise_and)
            V.tensor_scalar(out_my, mk16[:, 0:cb, :, 0::2], 0, None,
                            Alu.not_equal)
            V.tensor_scalar(out_op, mk16[:, 0:cb, :, 1::2], 0, None,
                            Alu.not_equal)

        # ---------- line group (ch8-12 / ch13-17) ----------
        # small chunks first so the DMA stream starts early
        LINE_CHUNKS = ((0, 4), (4, 4), (8, 8), (16, 8), (24, 8))
        CONN_CHUNKS = ((0, 8), (8, 8), (16, 8), (24, 4), (28, 4))
        for n0, cb in LINE_CHUNKS:
            hs = slice(n0, n0 + cb)
            outtL = epool.tile([P, CB, 5, 64], DT.float32, name="outtL")
            outtO = epool.tile([P, CB, 5, 64], DT.float32, name="outtO")
            outtL, outtO = outtL[:, 0:cb], outtO[:, 0:cb]
            expand(slice(3, 6), n0, cb, outtL[:, :, 0:3, :],
                   outtO[:, :, 0:3, :])
            A.activation(
                outtL[:, :, 3:5, :],
                dge[:, 0:2, hs].rearrange("p d n -> p n d")[:, :, :, None]
                .broadcast_to((P, cb, 2, 64)), Act.Copy)
            A.activation(
                outtO[:, :, 3:5, :],
                dge[:, 2:4, hs].rearrange("p d n -> p n d")[:, :, :, None]
                .broadcast_to((P, cb, 2, 64)), Act.Copy)
            nc.sync.dma_start(
                out_v[:, hs, 512:832],
                outtL.rearrange("p b c x -> p b (c x)"))
            nc.sync.dma_start(
                out_v[:, hs, 832:1152],
                outtO.rearrange("p b c x -> p b (c x)"))

        # ---------- connectivity (overlaps line-group DMA) ----------
        AV = pool.tile([P, 12, NB, 8], DT.uint32, name="AV")
        cx1 = pool.tile([P, NB, 8], DT.uint32, name="cx1")
        cx2 = pool.tile([P, NB, 8], DT.uint32, name="cx2")
        cx3 = pool.tile([P, NB, 8], DT.uint32, name="cx3")

        d2, d3, d4, t3, t4 = (CV[:, i] for i in range(5))
        mv = Ad[:, :, ROWS]
        for k, (di, dj) in enumerate(DIRS):
            def fwd(t):
                return t[:, :, R0 - di:R0 + 8 - di]

            def bwd(t, m=1):
                return t[:, :, R0 + m * di:R0 + 8 + m * di]

            a2, a3, a4 = (AV[:, 3 * k + i] for i in range(3))
            _stt(V, d2[:, :, ROWS], fwd(Ad), dj, Alu.bitwise_and, mv)
            _stt(V, d3[:, :, ROWS], fwd(d2), dj, Alu.bitwise_and,
                 d2[:, :, ROWS])
            _stt(V, d4[:, :, ROWS], fwd(d3), dj, Alu.bitwise_and,
                 d3[:, :, ROWS])
            _stt(V, a2, bwd(d2), -dj, Alu.bitwise_or, d2[:, :, ROWS])
            _stt(V, t3[:, :, ROWS], bwd(d3), -dj, Alu.bitwise_or,
                 d3[:, :, ROWS])
            _stt(V, a3, bwd(d3, 2), -2 * dj, Alu.bitwise_or, t3[:, :, ROWS])
            _stt(V, t4[:, :, ROWS], bwd(d4), -dj, Alu.bitwise_or,
                 d4[:, :, ROWS])
            _stt(V, a4, bwd(t4, 2), -2 * dj, Alu.bitwise_or, t4[:, :, ROWS])

        # conn merges (log trees over the dir-stacked AV lanes)
        V.tensor_tensor(cx1[:], AV[:, 0], AV[:, 3], Alu.bitwise_and)
        V.tensor_tensor(cx2[:], AV[:, 6], AV[:, 9], Alu.bitwise_and)
        V.tensor_tensor(cx1[:], cx1[:], cx2[:], Alu.bitwise_and)
        V.tensor_tensor(Rg[:, 0], mv, cx1[:], Alu.bitwise_xor)
        for kk, N in ((1, 2), (2, 3)):
            i0 = N - 2
            x4 = AV[:, i0::3]   # a_N for 4 dirs (lane-stride 3)
            y4 = AV[:, i0 + 1::3]
            V.tensor_tensor(x4, x4, y4, Alu.bitwise_xor)
            V.tensor_tensor(cx1[:], x4[:, 0], x4[:, 1], Alu.bitwise_or)
            V.tensor_tensor(cx2[:], x4[:, 2], x4[:, 3], Alu.bitwise_or)
            V.tensor_tensor(Rg[:, kk], cx1[:], cx2[:], Alu.bitwise_or)

        # ---------- conn group (ch2-7) ----------
        for n0, cb in CONN_CHUNKS:
            hs = slice(n0, n0 + cb)
            outtC = epool.tile([P, CB, 6, 64], DT.float32, name="outtC")
            outtC = outtC[:, 0:cb]
            expand(slice(0, 3), n0, cb, outtC[:, :, 0:3, :],
                   outtC[:, :, 3:6, :])
            nc.sync.dma_start(
                out_v[:, hs, 128:512],
                outtC.rearrange("p b c x -> p b (c x)"))


_NC_CACHE = None


def _build_nc():
    global _NC_CACHE
    if _NC_CACHE is not None:
        return _NC_CACHE
    nc = bacc.Bacc("TRN2", debug=False, enable_asserts=False)
    state_d = nc.dram_tensor("state", [BPC, 64], DT.float32, kind="ExternalInput").ap()
    side_d = nc.dram_tensor("side", [BPC], DT.float32, kind="ExternalInput").ap()
    out_d = nc.dram_tensor("out", [BPC, 18 * 64], DT.float32, kind="ExternalOutput").ap()
    with tile.TileContext(nc) as tc:
        feature_kernel(tc, out_d, state_d, side_d)
    nc.finalize()
    _NC_CACHE = nc
    return nc


_JIT_CACHE = None


def _get_runner():
    """Build a jitted shard_map runner over the 8 cores, fed with
    pre-sharded jax Arrays (avoids XLA-side resharding programs, which the
    neuron compiler chokes on for these sizes)."""
    global _JIT_CACHE
    if _JIT_CACHE is not None:
        return _JIT_CACHE
    import jax
    from jax.sharding import Mesh, PartitionSpec, NamedSharding
    try:
        from jax.experimental.shard_map import shard_map
    except ImportError:
        from jax.shard_map import shard_map  # newer jax
    from concourse import bass2jax as B2J

    B2J.install_neuronx_cc_hook()
    nc = _build_nc()

    in_names = ["state", "side"]
    out_names = ["out"]
    out_avals = [jax.core.ShapedArray((BPC, 18 * 64), np.float32)]
    all_names = in_names + out_names
    if nc.partition_id_tensor is not None:
        all_names = all_names + [nc.partition_id_tensor.name]

    def _body(state_a, side_a, zeros_a):
        operands = [state_a, side_a, zeros_a]
        if nc.partition_id_tensor is not None:
            operands.append(B2J.partition_id_tensor())
        outs = B2J._bass_exec_p.bind(
            *operands,
            out_avals=tuple(out_avals),
            in_names=tuple(all_names),
            out_names=tuple(out_names),
            lowering_input_output_aliases=(),
            sim_require_finite=True,
            sim_require_nnan=True,
            nc=nc,
        )
        return outs[0]

    devices = jax.devices()[:NCORES]
    mesh = Mesh(np.asarray(devices), ("core",))
    spec = PartitionSpec("core")
    sharded = jax.jit(
        shard_map(
            _body, mesh=mesh,
            in_specs=(spec, spec, spec),
            out_specs=spec,
            check_rep=False,
        ),
        donate_argnums=(2,),
        keep_unused=True,
    )

    def put(shards):
        arrs = [jax.device_put(s, devices[i]) for i, s in enumerate(shards)]
        global_shape = (sum(s.shape[0] for s in shards),) + shards[0].shape[1:]
        return jax.make_array_from_single_device_arrays(
            global_shape, NamedSharding(mesh, spec), arrs
        )

    _JIT_CACHE = (sharded, put)
    return _JIT_CACHE


def kernel(state, side):
    """Full-input entry point: state [32768,8,8] f32, side [32768] f32."""
    state = np.ascontiguousarray(np.asarray(state, dtype=np.float32)).reshape(-1, 64)
    side = np.ascontiguousarray(np.asarray(side, dtype=np.float32)).reshape(-1)
    B = state.shape[0]
    assert B == BPC * NCORES, (B, BPC * NCORES)
    sharded, put = _get_runner()
    state_g = put([state[i * BPC:(i + 1) * BPC] for i in range(NCORES)])
    side_g = put([side[i * BPC:(i + 1) * BPC] for i in range(NCORES)])
    zeros_g = put([np.zeros((BPC, 18 * 64), np.float32) for _ in range(NCORES)])
    out = sharded(state_g, side_g, zeros_g)
    out = np.asarray(out).reshape(NCORES * BPC, 18, 8, 8)
    return out


# revision 27
# speedup vs baseline: 1.0294x; 1.0294x over previous
"""v3 feature kernel: dual-player packed planes (my: bits 0-7, op: bits 16-23),
2-stage board pipeline so output DMA streams during logic of the next group.

Key changes vs v2:
- Expansion: per-j fused tensor_scalar ((Rg >> j|16+j) & 1 -> f32) straight
  from the packed channel planes; no mask tile, no mask-AND pass, no IS_NE
  pass. TS-class ops run at 2 partitions/cycle on DVE.
- Doubles: SWAR popcount on the packed words + log-step row adds; no f32
  tensor_reduce over expanded channels.
- Engine split: V = is_eq/pack/planes/line-row/conn dirs01/merges/SWAR/exp-my;
  GpSimd = line-col/conn dirs23/exp-op; Act = doubles broadcast.
- Output DMA per channel group (ch0-1 early, conn ch2-7, ch8-12, ch13-17)
  per pipe; 2 pipes of 16 boards pipeline logic with expansion DMA.
"""
import numpy as np

import concourse.bass as bass
import concourse.bacc as bacc
import concourse.mybir as mybir
import concourse.tile as tile

Alu = mybir.AluOpType
Act = mybir.ActivationFunctionType
DT = mybir.dt

P = 128
NB = 32
CB = 8             # boards per expansion chunk
NCORES = 8
BPC = P * NB
PAD = 18
R0 = 5
ROWS = slice(R0, R0 + 8)
SEG = 0x00FF00FF
PADT = 12
R0T = 2

DIRS = ((0, 1), (1, 0), (1, 1), (1, -1))


def _stt_raw(eng, out, in0, imm, in1, op0, op1, imm_dt=DT.uint32):
    outs = [eng.lower_ap(out)]
    return eng.add_instruction(
        mybir.InstTensorScalarPtr(
            name=eng.bass.get_next_instruction_name(),
            is_scalar_tensor_tensor=True,
            op0=op0, op1=op1,
            ins=[eng.lower_ap(in0),
                 mybir.ImmediateValue(dtype=imm_dt, value=imm),
                 eng.lower_ap(in1)],
            outs=outs,
        )
    )


def _stt(eng, out, in0, sh, op1, in1):
    if sh > 0:
        _stt_raw(eng, out, in0, sh, in1, Alu.logical_shift_left, op1)
    elif sh < 0:
        _stt_raw(eng, out, in0, -sh, in1, Alu.logical_shift_right, op1)
    else:
        eng.tensor_tensor(out, in0, in1, op1)


def feature_kernel(tc, out_d, state_d, side_d):
    nc = tc.nc
    V, A = nc.vector, nc.scalar

    state_v = state_d.rearrange("(p n) c -> p n c", p=P)
    side_v = side_d.rearrange("(p n) -> p n", p=P)
    out_v = out_d.rearrange("(p n) c -> p n c", p=P)

    with (
        tc.tile_pool(name="main", bufs=1) as pool,
        tc.tile_pool(name="exp", bufs=2) as epool,
    ):
        # ---------- input ----------
        sideT = pool.tile([P, NB], DT.float32, name="sideT")
        nc.sync.dma_start(sideT[:], side_v)

        mo = pool.tile([P, NB, 2, 64], DT.float32, name="mo")   # ch0/1 planes
        Rg = pool.tile([P, 6, NB, 8], DT.uint32, name="Rg")     # packed channels
        dge = pool.tile([P, 4, NB], DT.float32, name="dge")     # doubles flags
        myR = pool.tile([P, NB, 8], DT.uint32, name="myR")
        opR = pool.tile([P, NB, 8], DT.uint32, name="opR")

        # tiles whose guard-zeroing can overlap the input DMA
        P4 = pool.tile([P, 4, NB, PAD], DT.uint32, name="P4")
        V.memset(P4[:, :, :, 0:R0], 0)
        V.memset(P4[:, :, :, R0 + 8:PAD], 0)
        NAMES = ("t", "u", "a", "w", "b", "y", "q", "l2", "m3", "r1", "c",
                 "i1", "l3", "lb", "d0", "d1", "d", "j1", "md", "o3", "rb",
                 "x", "e", "g1", "g2", "r3", "c_l2", "c_l3", "c_r3")
        TMP = pool.tile([P, len(NAMES), NB, PADT], DT.uint32, name="TMP")
        V.memset(TMP[:, :, :, 0:R0T], 0)
        V.memset(TMP[:, :, :, R0T + 8:PADT], 0)
        lbm = pool.tile([P, NB, 8], DT.uint32, name="lbm")
        rbm = pool.tile([P, NB, 8], DT.uint32, name="rbm")
        V.memset(lbm[:], 0)
        V.memset(lbm[:, :, 0:1], SEG)
        V.memset(rbm[:], 0)
        V.memset(rbm[:, :, 3:8], SEG)
        CV = pool.tile([P, 5, NB, PAD], DT.uint32, name="CV")
        V.memset(CV[:, :, :, R0 - 1:R0], 0)
        V.memset(CV[:, :, :, R0 + 8:R0 + 10], 0)

        with tc.tile_pool(name="pre", bufs=1) as ppre:
            s = ppre.tile([P, NB, 64], DT.float32, name="s")
            nc.sync.dma_start(s[:], state_v)
            pk1 = ppre.tile([P, NB, 8, 4], DT.float32, name="pk1")
            pk2 = ppre.tile([P, NB, 8, 2], DT.float32, name="pk2")

            ss = ppre.tile([P, NB, 64], DT.float32, name="ss")
            V.tensor_tensor(
                ss[:], s[:],
                sideT[:, :, None].broadcast_to((P, NB, 64)), Alu.mult)
            A.activation(mo[:, :, 0], ss[:], Act.Relu)
            A.activation(mo[:, :, 1], ss[:], Act.Relu, 0.0, -1.0)
            nc.sync.dma_start(
                out_v[:, :, 0:128],
                mo.rearrange("p n c x -> p n (c x)"))

            def pack(dst_ap, srcf):
                v = srcf.rearrange("p n (r j2 t) -> p n r j2 t", t=2, j2=4)
                a1, b1 = v[:, :, :, :, 1], v[:, :, :, :, 0]
                V.scalar_tensor_tensor(pk1[:], a1, 2.0, b1, op0=Alu.mult,
                                       op1=Alu.add)
                ww2 = pk1.rearrange("p n r (k t) -> p n r k t", t=2)
                a2, b2 = ww2[:, :, :, :, 1], ww2[:, :, :, :, 0]
                V.scalar_tensor_tensor(pk2[:], a2, 4.0, b2, op0=Alu.mult,
                                       op1=Alu.add)
                a3, b3 = pk2[:, :, :, 1], pk2[:, :, :, 0]
                V.scalar_tensor_tensor(dst_ap, a3, 16.0, b3, op0=Alu.mult,
                                       op1=Alu.add)

            pack(myR[:], mo[:, :, 0])
            pack(opR[:], mo[:, :, 1])

        # ---------- planes ----------
        Ad, Bd, Ed, Nd = (P4[:, i] for i in range(4))
        _stt(V, Ad[:, :, ROWS], opR[:], 16, Alu.bitwise_or, myR[:])
        _stt(V, Bd[:, :, ROWS], myR[:], 16, Alu.bitwise_or, opR[:])
        V.tensor_tensor(Ed[:, :, ROWS], Ad[:, :, ROWS], Bd[:, :, ROWS],
                        Alu.bitwise_or)
        V.tensor_scalar(Ed[:, :, ROWS], Ed[:, :, ROWS], SEG, None,
                        Alu.bitwise_xor)
        V.tensor_scalar(Nd[:, :, ROWS], Ad[:, :, ROWS], SEG, None,
                        Alu.bitwise_xor)

        # line-feature tmps, shared between row and col mode (V-serial)
        nix = {n: i for i, n in enumerate(NAMES)}

        def TT(n, k=0):
            return TMP[:, nix[n], :, R0T + k:R0T + 8 + k]

        # ---------- line features (rows then cols, shared tmps) ----------
        def line_feats_row():
            me, op, em, nm = (x[:, :, ROWS] for x in (Ad, Bd, Ed, Nd))
            T = TT
            _stt(V, T("t"), me, -1, Alu.bitwise_and, me)
            _stt(V, T("u"), em, -1, Alu.bitwise_and, em)
            _stt(V, T("a"), T("u"), -2, Alu.bitwise_and, T("t"))
            _stt(V, T("w"), em, -3, Alu.bitwise_and, em)
            _stt(V, T("b"), T("t"), -1, Alu.bitwise_and, T("w"))
            _stt(V, T("y"), T("b"), 1, Alu.bitwise_or, T("b"))
            V.tensor_tensor(T("q"), T("a"), T("y"), Alu.bitwise_or)
            _stt(V, T("l2"), T("q"), 1, Alu.bitwise_or, T("a"))

            _stt(V, T("m3"), me, -2, Alu.bitwise_and, T("t"))
            _stt(V, T("r1"), em, -4, Alu.bitwise_and, em)
            _stt(V, T("c"), T("m3"), -1, Alu.bitwise_and, T("r1"))
            _stt(V, T("i1"), T("c"), 1, Alu.bitwise_or, T("c"))
            _stt(V, T("l3"), T("i1"), 1, Alu.bitwise_or, T("c"))

            V.tensor_scalar(T("lb"), op, 1, 0x00010001,
                            op0=Alu.logical_shift_left, op1=Alu.bitwise_or)
            _stt(V, T("d0"), em, -3, Alu.bitwise_and, T("m3"))
            _stt(V, T("d1"), nm, -4, Alu.bitwise_and, T("d0"))
            V.tensor_tensor(T("d"), T("d1"), T("lb"), Alu.bitwise_and)
            _stt(V, T("j1"), T("d"), 1, Alu.bitwise_or, T("d"))
            _stt(V, T("md"), T("d"), 2, Alu.bitwise_or, T("j1"))
            _stt(V, T("o3"), T("m3"), -1, Alu.bitwise_and, nm)
            _stt(V, T("o3"), nm, -4, Alu.bitwise_and, T("o3"))
            V.tensor_scalar(T("rb"), op, 5, 0x00F800F8,
                            op0=Alu.logical_shift_right, op1=Alu.bitwise_or)
            V.tensor_tensor(T("x"), T("lb"), T("rb"), Alu.bitwise_xor)
            V.tensor_tensor(T("e"), T("o3"), T("x"), Alu.bitwise_and)
            _stt(V, T("g1"), T("e"), 1, Alu.bitwise_or, T("e"))
            _stt(V, T("g2"), T("g1"), 1, Alu.bitwise_or, T("e"))
            _stt(V, T("r3"), T("g2"), 1, Alu.bitwise_or, T("md"))

        def line_feats_col():
            def dn(x, k):
                return x[:, :, R0 + k:R0 + 8 + k]

            def T(n, k=0):
                nm = "c_" + n if n in ("l2", "l3", "r3") else n
                return TT(nm, k)

            me, op, em, nm = Ad, Bd, Ed, Nd

            def MV(x, k=0):
                return x[:, :, R0 + k:R0 + 8 + k]

            V.tensor_tensor(T("t"), MV(me), dn(me, 1), Alu.bitwise_and)
            V.tensor_tensor(T("u"), MV(em), dn(em, 1), Alu.bitwise_and)
            V.tensor_tensor(T("a"), T("t"), T("u", 2), Alu.bitwise_and)
            V.tensor_tensor(T("w"), MV(em), dn(em, 3), Alu.bitwise_and)
            V.tensor_tensor(T("b"), T("w"), T("t", 1), Alu.bitwise_and)
            V.tensor_tensor(T("y"), T("b"), T("b", -1), Alu.bitwise_or)
            V.tensor_tensor(T("q"), T("a"), T("y"), Alu.bitwise_or)
            V.tensor_tensor(T("l2"), T("a"), T("q", -1), Alu.bitwise_or)

            V.tensor_tensor(T("m3"), T("t"), dn(me, 2), Alu.bitwise_and)
            V.tensor_tensor(T("r1"), MV(em), dn(em, 4), Alu.bitwise_and)
            V.tensor_tensor(T("c"), T("r1"), T("m3", 1), Alu.bitwise_and)
            V.tensor_tensor(T("i1"), T("c"), T("c", -1), Alu.bitwise_or)
            V.tensor_tensor(T("l3"), T("c"), T("i1", -1), Alu.bitwise_or)

            V.tensor_tensor(T("lb"), MV(op, -1), lbm[:], Alu.bitwise_or)
            V.tensor_tensor(T("d0"), T("m3"), dn(em, 3), Alu.bitwise_and)
            V.tensor_tensor(T("d1"), T("d0"), dn(nm, 4), Alu.bitwise_and)
            V.tensor_tensor(T("d"), T("d1"), T("lb"), Alu.bitwise_and)
            V.tensor_tensor(T("j1"), T("d"), T("d", -1), Alu.bitwise_or)
            V.tensor_tensor(T("md"), T("j1"), T("d", -2), Alu.bitwise_or)
            V.tensor_tensor(T("o3"), T("m3", 1), MV(nm), Alu.bitwise_and)
            V.tensor_tensor(T("o3"), T("o3"), dn(nm, 4), Alu.bitwise_and)
            V.tensor_tensor(T("rb"), MV(op, 5), rbm[:], Alu.bitwise_or)
            V.tensor_tensor(T("x"), T("lb"), T("rb"), Alu.bitwise_xor)
            V.tensor_tensor(T("e"), T("o3"), T("x"), Alu.bitwise_and)
            V.tensor_tensor(T("g1"), T("e"), T("e", -1), Alu.bitwise_or)
            V.tensor_tensor(T("g2"), T("e"), T("g1", -1), Alu.bitwise_or)
            V.tensor_tensor(T("r3"), T("md"), T("g2", -1), Alu.bitwise_or)

        line_feats_row()
        line_feats_col()

        V.tensor_tensor(Rg[:, 3], TT("l2"), TT("c_l2"), Alu.bitwise_or)
        _stt(V, Rg[:, 4], TT("l3"), 1, Alu.bitwise_or, TT("c_l3", -1))
        V.tensor_tensor(Rg[:, 5], TT("r3"), TT("c_r3"), Alu.bitwise_or)

        # ---------- doubles via SWAR popcount on packed planes ----------
        w1 = pool.tile([P, 3, NB, 8], DT.uint32, name="w1")
        w2 = pool.tile([P, 3, NB, 8], DT.uint32, name="w2")
        cs23 = pool.tile([P, 4, NB], DT.uint32, name="cs23")
        V.tensor_scalar(w1[:], Rg[:, 3:6], 1, 0x00550055,
                        op0=Alu.logical_shift_right, op1=Alu.bitwise_and)
        V.tensor_tensor(w1[:], Rg[:, 3:6], w1[:], Alu.subtract)
        V.tensor_scalar(w2[:], w1[:], 2, 0x00330033,
                        op0=Alu.logical_shift_right, op1=Alu.bitwise_and)
        V.tensor_scalar(w1[:], w1[:], 0x00330033, None, Alu.bitwise_and)
        V.tensor_tensor(w1[:], w1[:], w2[:], Alu.add)
        V.tensor_scalar(w2[:], w1[:], 4, None, Alu.logical_shift_right)
        V.tensor_tensor(w1[:], w1[:], w2[:], Alu.add)
        V.tensor_scalar(w1[:], w1[:], 0x000F000F, None, Alu.bitwise_and)
        V.tensor_tensor(w1[:, :, :, 0:4], w1[:, :, :, 0:4],
                        w1[:, :, :, 4:8], Alu.add)
        V.tensor_tensor(w1[:, :, :, 0:2], w1[:, :, :, 0:2],
                        w1[:, :, :, 2:4], Alu.add)
        V.tensor_tensor(cs23[:, 0:3], w1[:, :, :, 0], w1[:, :, :, 1],
                        Alu.add)
        V.tensor_tensor(cs23[:, 3], cs23[:, 1], cs23[:, 2], Alu.add)
        V.tensor_scalar(cs23[:, 1], cs23[:, 0], 0xFF, None, Alu.bitwise_and)
        V.tensor_scalar(cs23[:, 2], cs23[:, 3], 0xFF, None, Alu.bitwise_and)
        V.tensor_scalar(cs23[:, 0], cs23[:, 0], 16, None,
                        Alu.logical_shift_right)
        V.tensor_scalar(cs23[:, 3], cs23[:, 3], 16, None,
                        Alu.logical_shift_right)
        V.tensor_scalar(dge[:, 0], cs23[:, 1], 2, None, Alu.is_ge)
        V.tensor_scalar(dge[:, 1], cs23[:, 2], 2, None, Alu.is_ge)
        V.tensor_scalar(dge[:, 2], cs23[:, 0], 2, None, Alu.is_ge)
        V.tensor_scalar(dge[:, 3], cs23[:, 3], 2, None, Alu.is_ge)

        # ---------- expansion helpers ----------
        mk = pool.tile([P, CB, 3, 64], DT.uint32, name="mk")
        mk16 = mk[:].bitcast(DT.uint16)

        def expand(lanes, n0, cb, out_my, out_op):
            Rv = Rg[:, lanes, n0:n0 + cb].rearrange("p c n r -> p n c r")
            for j in range(8):
                V.tensor_scalar(mk[:, 0:cb, :, j::8], Rv, j, 0x00010001,
                                op0=Alu.logical_shift_right,
                                op1=Alu.bitwise_and)
            A.activation(out_my, mk16[:, 0:cb, :, 0::2], Act.Copy)
            A.activation(out_op, mk16[:, 0:cb, :, 1::2], Act.Copy)

        # ---------- line group (ch8-12 / ch13-17) ----------
        LINE_CHUNKS = tuple((k * CB, CB) for k in range(NB // CB))
        CONN_CHUNKS = LINE_CHUNKS
        for n0, cb in LINE_CHUNKS:
            hs = slice(n0, n0 + cb)
            outtL = epool.tile([P, CB, 5, 64], DT.float32, name="outtL")
            outtO = epool.tile([P, CB, 5, 64], DT.float32, name="outtO")
            outtL, outtO = outtL[:, 0:cb], outtO[:, 0:cb]
            expand(slice(3, 6), n0, cb, outtL[:, :, 0:3, :],
                   outtO[:, :, 0:3, :])
            A.activation(
                outtL[:, :, 3:5, :],
                dge[:, 0:2, hs].rearrange("p d n -> p n d")[:, :, :, None]
                .broadcast_to((P, cb, 2, 64)), Act.Copy)
            A.activation(
                outtO[:, :, 3:5, :],
                dge[:, 2:4, hs].rearrange("p d n -> p n d")[:, :, :, None]
                .broadcast_to((P, cb, 2, 64)), Act.Copy)
            nc.sync.dma_start(
                out_v[:, hs, 512:832],
                outtL.rearrange("p b c x -> p b (c x)"))
            nc.sync.dma_start(
                out_v[:, hs, 832:1152],
                outtO.rearrange("p b c x -> p b (c x)"))

        # ---------- connectivity (overlaps line-group DMA) ----------
        AV = pool.tile([P, 12, NB, 8], DT.uint32, name="AV")
        cx1 = pool.tile([P, NB, 8], DT.uint32, name="cx1")
        cx2 = pool.tile([P, NB, 8], DT.uint32, name="cx2")
        cx3 = pool.tile([P, NB, 8], DT.uint32, name="cx3")

        d2, d3, d4, t3, t4 = (CV[:, i] for i in range(5))
        mv = Ad[:, :, ROWS]
        for k, (di, dj) in enumerate(DIRS):
            def fwd(t):
                return t[:, :, R0 - di:R0 + 8 - di]

            def bwd(t, m=1):
                return t[:, :, R0 + m * di:R0 + 8 + m * di]

            a2, a3, a4 = (AV[:, 3 * k + i] for i in range(3))
            _stt(V, d2[:, :, ROWS], fwd(Ad), dj, Alu.bitwise_and, mv)
            _stt(V, d3[:, :, ROWS], fwd(d2), dj, Alu.bitwise_and,
                 d2[:, :, ROWS])
            _stt(V, d4[:, :, ROWS], fwd(d3), dj, Alu.bitwise_and,
                 d3[:, :, ROWS])
            _stt(V, a2, bwd(d2), -dj, Alu.bitwise_or, d2[:, :, ROWS])
            _stt(V, t3[:, :, ROWS], bwd(d3), -dj, Alu.bitwise_or,
                 d3[:, :, ROWS])
            _stt(V, a3, bwd(d3, 2), -2 * dj, Alu.bitwise_or, t3[:, :, ROWS])
            _stt(V, t4[:, :, ROWS], bwd(d4), -dj, Alu.bitwise_or,
                 d4[:, :, ROWS])
            _stt(V, a4, bwd(t4, 2), -2 * dj, Alu.bitwise_or, t4[:, :, ROWS])

        # conn merges (log trees over the dir-stacked AV lanes)
        V.tensor_tensor(cx1[:], AV[:, 0], AV[:, 3], Alu.bitwise_and)
        V.tensor_tensor(cx2[:], AV[:, 6], AV[:, 9], Alu.bitwise_and)
        V.tensor_tensor(cx1[:], cx1[:], cx2[:], Alu.bitwise_and)
        V.tensor_tensor(Rg[:, 0], mv, cx1[:], Alu.bitwise_xor)
        for kk, N in ((1, 2), (2, 3)):
            i0 = N - 2
            x4 = AV[:, i0::3]   # a_N for 4 dirs (lane-stride 3)
            y4 = AV[:, i0 + 1::3]
            V.tensor_tensor(x4, x4, y4, Alu.bitwise_xor)
            V.tensor_tensor(cx1[:], x4[:, 0], x4[:, 1], Alu.bitwise_or)
            V.tensor_tensor(cx2[:], x4[:, 2], x4[:, 3], Alu.bitwise_or)
            V.tensor_tensor(Rg[:, kk], cx1[:], cx2[:], Alu.bitwise_or)

        # ---------- conn group (ch2-7) ----------
        for n0, cb in CONN_CHUNKS:
            hs = slice(n0, n0 + cb)
            outtC = epool.tile([P, CB, 6, 64], DT.float32, name="outtC")
            outtC = outtC[:, 0:cb]
            expand(slice(0, 3), n0, cb, outtC[:, :, 0:3, :],
                   outtC[:, :, 3:6, :])
            nc.sync.dma_start(
                out_v[:, hs, 128:512],
                outtC.rearrange("p b c x -> p b (c x)"))


_NC_CACHE = None


def _build_nc():
    global _NC_CACHE
    if _NC_CACHE is not None:
        return _NC_CACHE
    nc = bacc.Bacc("TRN2", debug=False, enable_asserts=False)
    state_d = nc.dram_tensor("state", [BPC, 64], DT.float32, kind="ExternalInput").ap()
    side_d = nc.dram_tensor("side", [BPC], DT.float32, kind="ExternalInput").ap()
    out_d = nc.dram_tensor("out", [BPC, 18 * 64], DT.float32, kind="ExternalOutput").ap()
    with tile.TileContext(nc) as tc:
        feature_kernel(tc, out_d, state_d, side_d)
    nc.finalize()
    _NC_CACHE = nc
    return nc


_JIT_CACHE = None


def _get_runner():
    """Build a jitted shard_map runner over the 8 cores, fed with
    pre-sharded jax Arrays (avoids XLA-side resharding programs, which the
    neuron compiler chokes on for these sizes)."""
    global _JIT_CACHE
    if _JIT_CACHE is not None:
        return _JIT_CACHE
    import jax
    from jax.sharding import Mesh, PartitionSpec, NamedSharding
    try:
        from jax.experimental.shard_map import shard_map
    except ImportError:
        from jax.shard_map import shard_map  # newer jax
    from concourse import bass2jax as B2J

    B2J.install_neuronx_cc_hook()
    nc = _build_nc()

    in_names = ["state", "side"]
    out_names = ["out"]
    out_avals = [jax.core.ShapedArray((BPC, 18 * 64), np.float32)]
    all_names = in_names + out_names
    if nc.partition_id_tensor is not None:
        all_names = all_names + [nc.partition_id_tensor.name]

    def _body(state_a, side_a, zeros_a):
        operands = [state_a, side_a, zeros_a]
        if nc.partition_id_tensor is not None:
            operands.append(B2J.partition_id_tensor())
        outs = B2J._bass_exec_p.bind(
            *operands,
            out_avals=tuple(out_avals),
            in_names=tuple(all_names),
            out_names=tuple(out_names),
            lowering_input_output_aliases=(),
            sim_require_finite=True,
            sim_require_nnan=True,
            nc=nc,
        )
        return outs[0]

    devices = jax.devices()[:NCORES]
    mesh = Mesh(np.asarray(devices), ("core",))
    spec = PartitionSpec("core")
    sharded = jax.jit(
        shard_map(
            _body, mesh=mesh,
            in_specs=(spec, spec, spec),
            out_specs=spec,
            check_rep=False,
        ),
        donate_argnums=(2,),
        keep_unused=True,
    )

    def put(shards):
        arrs = [jax.device_put(s, devices[i]) for i, s in enumerate(shards)]
        global_shape = (sum(s.shape[0] for s in shards),) + shards[0].shape[1:]
        return jax.make_array_from_single_device_arrays(
            global_shape, NamedSharding(mesh, spec), arrs
        )

    _JIT_CACHE = (sharded, put)
    return _JIT_CACHE


def kernel(state, side):
    """Full-input entry point: state [32768,8,8] f32, side [32768] f32."""
    state = np.ascontiguousarray(np.asarray(state, dtype=np.float32)).reshape(-1, 64)
    side = np.ascontiguousarray(np.asarray(side, dtype=np.float32)).reshape(-1)
    B = state.shape[0]
    assert B == BPC * NCORES, (B, BPC * NCORES)
    sharded, put = _get_runner()
    state_g = put([state[i * BPC:(i + 1) * BPC] for i in range(NCORES)])
    side_g = put([side[i * BPC:(i + 1) * BPC] for i in range(NCORES)])
    zeros_g = put([np.zeros((BPC, 18 * 64), np.float32) for _ in range(NCORES)])
    out = sharded(state_g, side_g, zeros_g)
    out = np.asarray(out).reshape(NCORES * BPC, 18, 8, 8)
    return out


# revision 28
# speedup vs baseline: 1.0495x; 1.0195x over previous
"""v3 feature kernel: dual-player packed planes (my: bits 0-7, op: bits 16-23),
2-stage board pipeline so output DMA streams during logic of the next group.

Key changes vs v2:
- Expansion: per-j fused tensor_scalar ((Rg >> j|16+j) & 1 -> f32) straight
  from the packed channel planes; no mask tile, no mask-AND pass, no IS_NE
  pass. TS-class ops run at 2 partitions/cycle on DVE.
- Doubles: SWAR popcount on the packed words + log-step row adds; no f32
  tensor_reduce over expanded channels.
- Engine split: V = is_eq/pack/planes/line-row/conn dirs01/merges/SWAR/exp-my;
  GpSimd = line-col/conn dirs23/exp-op; Act = doubles broadcast.
- Output DMA per channel group (ch0-1 early, conn ch2-7, ch8-12, ch13-17)
  per pipe; 2 pipes of 16 boards pipeline logic with expansion DMA.
"""
import numpy as np

import concourse.bass as bass
import concourse.bacc as bacc
import concourse.mybir as mybir
import concourse.tile as tile

Alu = mybir.AluOpType
Act = mybir.ActivationFunctionType
DT = mybir.dt

P = 128
NB = 32
CB = 8             # boards per expansion chunk
NCORES = 8
BPC = P * NB
PAD = 18
R0 = 5
ROWS = slice(R0, R0 + 8)
SEG = 0x00FF00FF
PADT = 12
R0T = 2

DIRS = ((0, 1), (1, 0), (1, 1), (1, -1))


def _stt_raw(eng, out, in0, imm, in1, op0, op1, imm_dt=DT.uint32):
    outs = [eng.lower_ap(out)]
    return eng.add_instruction(
        mybir.InstTensorScalarPtr(
            name=eng.bass.get_next_instruction_name(),
            is_scalar_tensor_tensor=True,
            op0=op0, op1=op1,
            ins=[eng.lower_ap(in0),
                 mybir.ImmediateValue(dtype=imm_dt, value=imm),
                 eng.lower_ap(in1)],
            outs=outs,
        )
    )


def _stt(eng, out, in0, sh, op1, in1):
    if sh > 0:
        _stt_raw(eng, out, in0, sh, in1, Alu.logical_shift_left, op1)
    elif sh < 0:
        _stt_raw(eng, out, in0, -sh, in1, Alu.logical_shift_right, op1)
    else:
        eng.tensor_tensor(out, in0, in1, op1)


def feature_kernel(tc, out_d, state_d, side_d):
    nc = tc.nc
    V, A = nc.vector, nc.scalar

    state_v = state_d.rearrange("(p n) c -> p n c", p=P)
    side_v = side_d.rearrange("(p n) -> p n", p=P)
    out_v = out_d.rearrange("(p n) c -> p n c", p=P)

    with (
        tc.tile_pool(name="main", bufs=1) as pool,
        tc.tile_pool(name="exp", bufs=2) as epool,
    ):
        # ---------- input ----------
        sideT = pool.tile([P, NB], DT.float32, name="sideT")
        nc.sync.dma_start(sideT[:], side_v)

        mo = pool.tile([P, NB, 2, 64], DT.float32, name="mo")   # ch0/1 planes
        Rg = pool.tile([P, 6, NB, 8], DT.uint32, name="Rg")     # packed channels
        dge = pool.tile([P, 4, NB], DT.float32, name="dge")     # doubles flags
        myR = pool.tile([P, NB, 8], DT.uint32, name="myR")
        opR = pool.tile([P, NB, 8], DT.uint32, name="opR")

        # tiles whose guard-zeroing can overlap the input DMA
        P4 = pool.tile([P, 4, NB, PAD], DT.uint32, name="P4")
        V.memset(P4[:, :, :, 0:R0], 0)
        V.memset(P4[:, :, :, R0 + 8:PAD], 0)
        NAMES = ("t", "u", "a", "w", "b", "y", "q", "l2", "m3", "r1", "c",
                 "i1", "l3", "lb", "d0", "d1", "d", "j1", "md", "o3", "rb",
                 "x", "e", "g1", "g2", "r3", "c_l2", "c_l3", "c_r3")
        TMP = pool.tile([P, len(NAMES), NB, PADT], DT.uint32, name="TMP")
        V.memset(TMP[:, :, :, 0:R0T], 0)
        V.memset(TMP[:, :, :, R0T + 8:PADT], 0)
        lbm = pool.tile([P, NB, 8], DT.uint32, name="lbm")
        rbm = pool.tile([P, NB, 8], DT.uint32, name="rbm")
        V.memset(lbm[:], 0)
        V.memset(lbm[:, :, 0:1], SEG)
        V.memset(rbm[:], 0)
        V.memset(rbm[:, :, 3:8], SEG)
        CV = pool.tile([P, 5, NB, PAD], DT.uint32, name="CV")
        V.memset(CV[:, :, :, R0 - 1:R0], 0)
        V.memset(CV[:, :, :, R0 + 8:R0 + 10], 0)

        with tc.tile_pool(name="pre", bufs=1) as ppre:
            s = ppre.tile([P, NB, 64], DT.float32, name="s")
            nc.sync.dma_start(s[:], state_v)
            pk1 = ppre.tile([P, NB, 8, 4], DT.float32, name="pk1")
            pk2 = ppre.tile([P, NB, 8, 2], DT.float32, name="pk2")

            ss = ppre.tile([P, NB, 64], DT.float32, name="ss")
            V.tensor_tensor(
                ss[:], s[:],
                sideT[:, :, None].broadcast_to((P, NB, 64)), Alu.mult)
            A.activation(mo[:, :, 0], ss[:], Act.Relu)
            A.activation(mo[:, :, 1], ss[:], Act.Relu, 0.0, -1.0)
            nc.sync.dma_start(
                out_v[:, :, 0:128],
                mo.rearrange("p n c x -> p n (c x)"))

            def pack(dst_ap, srcf):
                v = srcf.rearrange("p n (r j2 t) -> p n r j2 t", t=2, j2=4)
                a1, b1 = v[:, :, :, :, 1], v[:, :, :, :, 0]
                V.scalar_tensor_tensor(pk1[:], a1, 2.0, b1, op0=Alu.mult,
                                       op1=Alu.add)
                ww2 = pk1.rearrange("p n r (k t) -> p n r k t", t=2)
                a2, b2 = ww2[:, :, :, :, 1], ww2[:, :, :, :, 0]
                V.scalar_tensor_tensor(pk2[:], a2, 4.0, b2, op0=Alu.mult,
                                       op1=Alu.add)
                a3, b3 = pk2[:, :, :, 1], pk2[:, :, :, 0]
                V.scalar_tensor_tensor(dst_ap, a3, 16.0, b3, op0=Alu.mult,
                                       op1=Alu.add)

            pack(myR[:], mo[:, :, 0])
            pack(opR[:], mo[:, :, 1])

        # ---------- planes ----------
        Ad, Bd, Ed, Nd = (P4[:, i] for i in range(4))
        _stt(V, Ad[:, :, ROWS], opR[:], 16, Alu.bitwise_or, myR[:])
        _stt(V, Bd[:, :, ROWS], myR[:], 16, Alu.bitwise_or, opR[:])
        V.tensor_tensor(Ed[:, :, ROWS], Ad[:, :, ROWS], Bd[:, :, ROWS],
                        Alu.bitwise_or)
        V.tensor_scalar(Ed[:, :, ROWS], Ed[:, :, ROWS], SEG, None,
                        Alu.bitwise_xor)
        V.tensor_scalar(Nd[:, :, ROWS], Ad[:, :, ROWS], SEG, None,
                        Alu.bitwise_xor)

        # line-feature tmps, shared between row and col mode (V-serial)
        nix = {n: i for i, n in enumerate(NAMES)}

        def TT(n, k=0):
            return TMP[:, nix[n], :, R0T + k:R0T + 8 + k]

        # ---------- line features (rows then cols, shared tmps) ----------
        def line_feats_row():
            me, op, em, nm = (x[:, :, ROWS] for x in (Ad, Bd, Ed, Nd))
            T = TT
            _stt(V, T("t"), me, -1, Alu.bitwise_and, me)
            _stt(V, T("u"), em, -1, Alu.bitwise_and, em)
            _stt(V, T("a"), T("u"), -2, Alu.bitwise_and, T("t"))
            _stt(V, T("w"), em, -3, Alu.bitwise_and, em)
            _stt(V, T("b"), T("t"), -1, Alu.bitwise_and, T("w"))
            _stt(V, T("y"), T("b"), 1, Alu.bitwise_or, T("b"))
            V.tensor_tensor(T("q"), T("a"), T("y"), Alu.bitwise_or)
            _stt(V, T("l2"), T("q"), 1, Alu.bitwise_or, T("a"))

            _stt(V, T("m3"), me, -2, Alu.bitwise_and, T("t"))
            _stt(V, T("r1"), em, -4, Alu.bitwise_and, em)
            _stt(V, T("c"), T("m3"), -1, Alu.bitwise_and, T("r1"))
            _stt(V, T("i1"), T("c"), 1, Alu.bitwise_or, T("c"))
            _stt(V, T("l3"), T("i1"), 1, Alu.bitwise_or, T("c"))

            V.tensor_scalar(T("lb"), op, 1, 0x00010001,
                            op0=Alu.logical_shift_left, op1=Alu.bitwise_or)
            _stt(V, T("d0"), em, -3, Alu.bitwise_and, T("m3"))
            _stt(V, T("d1"), nm, -4, Alu.bitwise_and, T("d0"))
            V.tensor_tensor(T("d"), T("d1"), T("lb"), Alu.bitwise_and)
            _stt(V, T("j1"), T("d"), 1, Alu.bitwise_or, T("d"))
            _stt(V, T("md"), T("d"), 2, Alu.bitwise_or, T("j1"))
            _stt(V, T("o3"), T("m3"), -1, Alu.bitwise_and, nm)
            _stt(V, T("o3"), nm, -4, Alu.bitwise_and, T("o3"))
            V.tensor_scalar(T("rb"), op, 5, 0x00F800F8,
                            op0=Alu.logical_shift_right, op1=Alu.bitwise_or)
            V.tensor_tensor(T("x"), T("lb"), T("rb"), Alu.bitwise_xor)
            V.tensor_tensor(T("e"), T("o3"), T("x"), Alu.bitwise_and)
            _stt(V, T("g1"), T("e"), 1, Alu.bitwise_or, T("e"))
            _stt(V, T("g2"), T("g1"), 1, Alu.bitwise_or, T("e"))
            _stt(V, T("r3"), T("g2"), 1, Alu.bitwise_or, T("md"))

        def line_feats_col():
            def dn(x, k):
                return x[:, :, R0 + k:R0 + 8 + k]

            def T(n, k=0):
                nm = "c_" + n if n in ("l2", "l3", "r3") else n
                return TT(nm, k)

            me, op, em, nm = Ad, Bd, Ed, Nd

            def MV(x, k=0):
                return x[:, :, R0 + k:R0 + 8 + k]

            V.tensor_tensor(T("t"), MV(me), dn(me, 1), Alu.bitwise_and)
            V.tensor_tensor(T("u"), MV(em), dn(em, 1), Alu.bitwise_and)
            V.tensor_tensor(T("a"), T("t"), T("u", 2), Alu.bitwise_and)
            V.tensor_tensor(T("w"), MV(em), dn(em, 3), Alu.bitwise_and)
            V.tensor_tensor(T("b"), T("w"), T("t", 1), Alu.bitwise_and)
            V.tensor_tensor(T("y"), T("b"), T("b", -1), Alu.bitwise_or)
            V.tensor_tensor(T("q"), T("a"), T("y"), Alu.bitwise_or)
            V.tensor_tensor(T("l2"), T("a"), T("q", -1), Alu.bitwise_or)

            V.tensor_tensor(T("m3"), T("t"), dn(me, 2), Alu.bitwise_and)
            V.tensor_tensor(T("r1"), MV(em), dn(em, 4), Alu.bitwise_and)
            V.tensor_tensor(T("c"), T("r1"), T("m3", 1), Alu.bitwise_and)
            V.tensor_tensor(T("i1"), T("c"), T("c", -1), Alu.bitwise_or)
            V.tensor_tensor(T("l3"), T("c"), T("i1", -1), Alu.bitwise_or)

            V.tensor_tensor(T("lb"), MV(op, -1), lbm[:], Alu.bitwise_or)
            V.tensor_tensor(T("d0"), T("m3"), dn(em, 3), Alu.bitwise_and)
            V.tensor_tensor(T("d1"), T("d0"), dn(nm, 4), Alu.bitwise_and)
            V.tensor_tensor(T("d"), T("d1"), T("lb"), Alu.bitwise_and)
            V.tensor_tensor(T("j1"), T("d"), T("d", -1), Alu.bitwise_or)
            V.tensor_tensor(T("md"), T("j1"), T("d", -2), Alu.bitwise_or)
            V.tensor_tensor(T("o3"), T("m3", 1), MV(nm), Alu.bitwise_and)
            V.tensor_tensor(T("o3"), T("o3"), dn(nm, 4), Alu.bitwise_and)
            V.tensor_tensor(T("rb"), MV(op, 5), rbm[:], Alu.bitwise_or)
            V.tensor_tensor(T("x"), T("lb"), T("rb"), Alu.bitwise_xor)
            V.tensor_tensor(T("e"), T("o3"), T("x"), Alu.bitwise_and)
            V.tensor_tensor(T("g1"), T("e"), T("e", -1), Alu.bitwise_or)
            V.tensor_tensor(T("g2"), T("e"), T("g1", -1), Alu.bitwise_or)
            V.tensor_tensor(T("r3"), T("md"), T("g2", -1), Alu.bitwise_or)

        line_feats_row()
        line_feats_col()

        V.tensor_tensor(Rg[:, 3], TT("l2"), TT("c_l2"), Alu.bitwise_or)
        _stt(V, Rg[:, 4], TT("l3"), 1, Alu.bitwise_or, TT("c_l3", -1))
        V.tensor_tensor(Rg[:, 5], TT("r3"), TT("c_r3"), Alu.bitwise_or)

        # ---------- doubles via SWAR popcount on packed planes ----------
        w1 = pool.tile([P, 3, NB, 8], DT.uint32, name="w1")
        w2 = pool.tile([P, 3, NB, 8], DT.uint32, name="w2")
        cs23 = pool.tile([P, 4, NB], DT.uint32, name="cs23")
        V.tensor_scalar(w1[:], Rg[:, 3:6], 1, 0x00550055,
                        op0=Alu.logical_shift_right, op1=Alu.bitwise_and)
        V.tensor_tensor(w1[:], Rg[:, 3:6], w1[:], Alu.subtract)
        V.tensor_scalar(w2[:], w1[:], 2, 0x00330033,
                        op0=Alu.logical_shift_right, op1=Alu.bitwise_and)
        V.tensor_scalar(w1[:], w1[:], 0x00330033, None, Alu.bitwise_and)
        V.tensor_tensor(w1[:], w1[:], w2[:], Alu.add)
        V.tensor_scalar(w2[:], w1[:], 4, None, Alu.logical_shift_right)
        V.tensor_tensor(w1[:], w1[:], w2[:], Alu.add)
        V.tensor_scalar(w1[:], w1[:], 0x000F000F, None, Alu.bitwise_and)
        V.tensor_tensor(w1[:, :, :, 0:4], w1[:, :, :, 0:4],
                        w1[:, :, :, 4:8], Alu.add)
        V.tensor_tensor(w1[:, :, :, 0:2], w1[:, :, :, 0:2],
                        w1[:, :, :, 2:4], Alu.add)
        V.tensor_tensor(cs23[:, 0:3], w1[:, :, :, 0], w1[:, :, :, 1],
                        Alu.add)
        V.tensor_tensor(cs23[:, 3], cs23[:, 1], cs23[:, 2], Alu.add)
        V.tensor_scalar(cs23[:, 1], cs23[:, 0], 0xFF, None, Alu.bitwise_and)
        V.tensor_scalar(cs23[:, 2], cs23[:, 3], 0xFF, None, Alu.bitwise_and)
        V.tensor_scalar(cs23[:, 0], cs23[:, 0], 16, None,
                        Alu.logical_shift_right)
        V.tensor_scalar(cs23[:, 3], cs23[:, 3], 16, None,
                        Alu.logical_shift_right)
        V.tensor_scalar(dge[:, 0], cs23[:, 1], 2, None, Alu.is_ge)
        V.tensor_scalar(dge[:, 1], cs23[:, 2], 2, None, Alu.is_ge)
        V.tensor_scalar(dge[:, 2], cs23[:, 0], 2, None, Alu.is_ge)
        V.tensor_scalar(dge[:, 3], cs23[:, 3], 2, None, Alu.is_ge)

        # ---------- expansion helpers ----------
        mk = pool.tile([P, CB, 3, 64], DT.uint32, name="mk")
        mk16 = mk[:].bitcast(DT.uint16)

        def expand(lanes, n0, cb, out_my, out_op, ne_eng="A"):
            Rv = Rg[:, lanes, n0:n0 + cb].rearrange("p c n r -> p n c r")
            for j in range(8):
                V.tensor_scalar(mk[:, 0:cb, :, j::8], Rv, j, 0x00010001,
                                op0=Alu.logical_shift_right,
                                op1=Alu.bitwise_and)
            if ne_eng == "A":
                A.activation(out_my, mk16[:, 0:cb, :, 0::2], Act.Copy)
                A.activation(out_op, mk16[:, 0:cb, :, 1::2], Act.Copy)
            else:
                V.tensor_scalar(out_my, mk16[:, 0:cb, :, 0::2], 0, None,
                                Alu.not_equal)
                V.tensor_scalar(out_op, mk16[:, 0:cb, :, 1::2], 0, None,
                                Alu.not_equal)

        # ---------- line group (ch8-12 / ch13-17) ----------
        LINE_CHUNKS = tuple((k * CB, CB) for k in range(NB // CB))
        CONN_CHUNKS = LINE_CHUNKS
        for n0, cb in LINE_CHUNKS:
            hs = slice(n0, n0 + cb)
            outtL = epool.tile([P, CB, 5, 64], DT.float32, name="outtL")
            outtO = epool.tile([P, CB, 5, 64], DT.float32, name="outtO")
            outtL, outtO = outtL[:, 0:cb], outtO[:, 0:cb]
            expand(slice(3, 6), n0, cb, outtL[:, :, 0:3, :],
                   outtO[:, :, 0:3, :])
            A.activation(
                outtL[:, :, 3:5, :],
                dge[:, 0:2, hs].rearrange("p d n -> p n d")[:, :, :, None]
                .broadcast_to((P, cb, 2, 64)), Act.Copy)
            A.activation(
                outtO[:, :, 3:5, :],
                dge[:, 2:4, hs].rearrange("p d n -> p n d")[:, :, :, None]
                .broadcast_to((P, cb, 2, 64)), Act.Copy)
            nc.sync.dma_start(
                out_v[:, hs, 512:832],
                outtL.rearrange("p b c x -> p b (c x)"))
            nc.sync.dma_start(
                out_v[:, hs, 832:1152],
                outtO.rearrange("p b c x -> p b (c x)"))

        # ---------- connectivity (overlaps line-group DMA) ----------
        AV = pool.tile([P, 12, NB, 8], DT.uint32, name="AV")
        cx1 = pool.tile([P, NB, 8], DT.uint32, name="cx1")
        cx2 = pool.tile([P, NB, 8], DT.uint32, name="cx2")
        cx3 = pool.tile([P, NB, 8], DT.uint32, name="cx3")

        d2, d3, d4, t3, t4 = (CV[:, i] for i in range(5))
        mv = Ad[:, :, ROWS]
        for k, (di, dj) in enumerate(DIRS):
            def fwd(t):
                return t[:, :, R0 - di:R0 + 8 - di]

            def bwd(t, m=1):
                return t[:, :, R0 + m * di:R0 + 8 + m * di]

            a2, a3, a4 = (AV[:, 3 * k + i] for i in range(3))
            _stt(V, d2[:, :, ROWS], fwd(Ad), dj, Alu.bitwise_and, mv)
            _stt(V, d3[:, :, ROWS], fwd(d2), dj, Alu.bitwise_and,
                 d2[:, :, ROWS])
            _stt(V, d4[:, :, ROWS], fwd(d3), dj, Alu.bitwise_and,
                 d3[:, :, ROWS])
            _stt(V, a2, bwd(d2), -dj, Alu.bitwise_or, d2[:, :, ROWS])
            _stt(V, t3[:, :, ROWS], bwd(d3), -dj, Alu.bitwise_or,
                 d3[:, :, ROWS])
            _stt(V, a3, bwd(d3, 2), -2 * dj, Alu.bitwise_or, t3[:, :, ROWS])
            _stt(V, t4[:, :, ROWS], bwd(d4), -dj, Alu.bitwise_or,
                 d4[:, :, ROWS])
            _stt(V, a4, bwd(t4, 2), -2 * dj, Alu.bitwise_or, t4[:, :, ROWS])

        # conn merges (log trees over the dir-stacked AV lanes)
        V.tensor_tensor(cx1[:], AV[:, 0], AV[:, 3], Alu.bitwise_and)
        V.tensor_tensor(cx2[:], AV[:, 6], AV[:, 9], Alu.bitwise_and)
        V.tensor_tensor(cx1[:], cx1[:], cx2[:], Alu.bitwise_and)
        V.tensor_tensor(Rg[:, 0], mv, cx1[:], Alu.bitwise_xor)
        for kk, N in ((1, 2), (2, 3)):
            i0 = N - 2
            x4 = AV[:, i0::3]   # a_N for 4 dirs (lane-stride 3)
            y4 = AV[:, i0 + 1::3]
            V.tensor_tensor(x4, x4, y4, Alu.bitwise_xor)
            V.tensor_tensor(cx1[:], x4[:, 0], x4[:, 1], Alu.bitwise_or)
            V.tensor_tensor(cx2[:], x4[:, 2], x4[:, 3], Alu.bitwise_or)
            V.tensor_tensor(Rg[:, kk], cx1[:], cx2[:], Alu.bitwise_or)

        # ---------- conn group (ch2-7) ----------
        for n0, cb in CONN_CHUNKS:
            hs = slice(n0, n0 + cb)
            outtC = epool.tile([P, CB, 6, 64], DT.float32, name="outtC")
            outtC = outtC[:, 0:cb]
            expand(slice(0, 3), n0, cb, outtC[:, :, 0:3, :],
                   outtC[:, :, 3:6, :], ne_eng="V")
            nc.sync.dma_start(
                out_v[:, hs, 128:512],
                outtC.rearrange("p b c x -> p b (c x)"))


_NC_CACHE = None


def _build_nc():
    global _NC_CACHE
    if _NC_CACHE is not None:
        return _NC_CACHE
    nc = bacc.Bacc("TRN2", debug=False, enable_asserts=False)
    state_d = nc.dram_tensor("state", [BPC, 64], DT.float32, kind="ExternalInput").ap()
    side_d = nc.dram_tensor("side", [BPC], DT.float32, kind="ExternalInput").ap()
    out_d = nc.dram_tensor("out", [BPC, 18 * 64], DT.float32, kind="ExternalOutput").ap()
    with tile.TileContext(nc) as tc:
        feature_kernel(tc, out_d, state_d, side_d)
    nc.finalize()
    _NC_CACHE = nc
    return nc


_JIT_CACHE = None


def _get_runner():
    """Build a jitted shard_map runner over the 8 cores, fed with
    pre-sharded jax Arrays (avoids XLA-side resharding programs, which the
    neuron compiler chokes on for these sizes)."""
    global _JIT_CACHE
    if _JIT_CACHE is not None:
        return _JIT_CACHE
    import jax
    from jax.sharding import Mesh, PartitionSpec, NamedSharding
    try:
        from jax.experimental.shard_map import shard_map
    except ImportError:
        from jax.shard_map import shard_map  # newer jax
    from concourse import bass2jax as B2J

    B2J.install_neuronx_cc_hook()
    nc = _build_nc()

    in_names = ["state", "side"]
    out_names = ["out"]
    out_avals = [jax.core.ShapedArray((BPC, 18 * 64), np.float32)]
    all_names = in_names + out_names
    if nc.partition_id_tensor is not None:
        all_names = all_names + [nc.partition_id_tensor.name]

    def _body(state_a, side_a, zeros_a):
        operands = [state_a, side_a, zeros_a]
        if nc.partition_id_tensor is not None:
            operands.append(B2J.partition_id_tensor())
        outs = B2J._bass_exec_p.bind(
            *operands,
            out_avals=tuple(out_avals),
            in_names=tuple(all_names),
            out_names=tuple(out_names),
            lowering_input_output_aliases=(),
            sim_require_finite=True,
            sim_require_nnan=True,
            nc=nc,
        )
        return outs[0]

    devices = jax.devices()[:NCORES]
    mesh = Mesh(np.asarray(devices), ("core",))
    spec = PartitionSpec("core")
    sharded = jax.jit(
        shard_map(
            _body, mesh=mesh,
            in_specs=(spec, spec, spec),
            out_specs=spec,
            check_rep=False,
        ),
        donate_argnums=(2,),
        keep_unused=True,
    )

    def put(shards):
        arrs = [jax.device_put(s, devices[i]) for i, s in enumerate(shards)]
        global_shape = (sum(s.shape[0] for s in shards),) + shards[0].shape[1:]
        return jax.make_array_from_single_device_arrays(
            global_shape, NamedSharding(mesh, spec), arrs
        )

    _JIT_CACHE = (sharded, put)
    return _JIT_CACHE


def kernel(state, side):
    """Full-input entry point: state [32768,8,8] f32, side [32768] f32."""
    state = np.ascontiguousarray(np.asarray(state, dtype=np.float32)).reshape(-1, 64)
    side = np.ascontiguousarray(np.asarray(side, dtype=np.float32)).reshape(-1)
    B = state.shape[0]
    assert B == BPC * NCORES, (B, BPC * NCORES)
    sharded, put = _get_runner()
    state_g = put([state[i * BPC:(i + 1) * BPC] for i in range(NCORES)])
    side_g = put([side[i * BPC:(i + 1) * BPC] for i in range(NCORES)])
    zeros_g = put([np.zeros((BPC, 18 * 64), np.float32) for _ in range(NCORES)])
    out = sharded(state_g, side_g, zeros_g)
    out = np.asarray(out).reshape(NCORES * BPC, 18, 8, 8)
    return out
